# revision 25
# baseline (speedup 1.0000x reference)
"""Multi-head cross-attention Bass/Tile kernel for Trainium2, SPMD over 8 cores.

Problem: B=4, SQ=1024, SK=2048, E=1024, H=16 heads, Dh=64.
  q = query @ Wq.T + bq ; k = key @ Wk.T + bk ; v = value @ Wv.T + bv
  attn = softmax(SCALE * q k^T) ; attended = attn @ v
  output = attended @ Wo.T + bo ; mean_attn = attn.mean(heads)

Sharding: 8 cores = (batch, query-half) shards. Each core owns one batch and
512 query rows, computes ALL heads for that shard, so both outputs are disjoint
slices -> host unshard is pure concatenation (no reduction).

Per-core pipeline (matmuls in fp32r = full-rate TF32-like, ~4e-4 rel err):
  P1: qT[d,sq] = (SCALE*Wq) @ queryT (+SCALE*bq)   [SBUF resident, 8 chunk tiles]
  P2: kT[d,sk] = Wk @ keyT (+bk)                   [SBUF resident, 8 chunk tiles]
  P3: v[sk,d]  = (valueT stationary) @ WvT         [DRAM spill, 2 d-half tensors]
  Per head pair p (heads 2p, 2p+1 ride distinct PE row groups -> concurrent):
    transposed pass: scoresT[sk,sq] -> exp (ACT, PSUM src) -> attendedT
      accumulated over sk chunks (PE)
    natural pass: scores[sq,sk] -> exp with accum_out = row sums (ACT)
      -> 1/denom (DVE) ; mean_acc[j] += exp * 1/(16*denom) (DVE fused STT)
    attendedT *= 1/denom (PE-transposed denoms -> DRAM row bcast -> DVE mul)
  output = attendedT^T @ WoT (+ bo + Wo@bv folded host-side)

Chunk-granular tiles (kT/qT per d-chunk, vd per d-half) let the Tile scheduler
overlap head-pair attention with the tail of the projections.

Self-contained: hardcodes shapes; no sibling imports.
"""

import numpy as np

import concourse.bass as bass
import concourse.tile as tile
from concourse import bacc, mybir
from concourse.bass_utils import run_bass_kernel_spmd
from concourse.masks import make_identity

F32 = mybir.dt.float32
F32R = mybir.dt.float32r
AX = mybir.AxisListType
ALU = mybir.AluOpType
ACTF = mybir.ActivationFunctionType

B, SQ, SK, E, H, DH = 4, 1024, 2048, 1024, 16, 64
SCALE = DH ** -0.5
NCORES = 8
SQL = SQ // 2           # 512 query rows per core
EC = E // 128           # 8 embed chunks
DC = E // 128           # 8 d chunks (qkv output dim = E)
NP = H // 2             # 8 head pairs
SKT = SK // 128         # 16 sk tiles of 128
SKT5 = SK // 512        # 4 sk tiles of 512
SQJ = SQL // 128        # 4 sq tiles of 128


def bcast_rows(src_row_ap, nrows):
    """AP reading one DRAM row broadcast across nrows partitions."""
    return bass.AP(src_row_ap.tensor, src_row_ap.offset,
                   [[0, nrows]] + list(src_row_ap.ap[1:]))


def build_kernel(with_bias: bool):
    nc = bacc.Bacc("TRN2", target_bir_lowering=False, debug=False)

    # ---- DRAM I/O ----
    xq_d = nc.dram_tensor("xq", [E, SQL], F32R, kind="ExternalInput").ap()
    xk_d = nc.dram_tensor("xk", [E, SK], F32R, kind="ExternalInput").ap()
    xv_d = nc.dram_tensor("xv", [E, SK], F32R, kind="ExternalInput").ap()
    wq_d = nc.dram_tensor("wq", [E, E], F32R, kind="ExternalInput").ap()
    wk_d = nc.dram_tensor("wk", [E, E], F32R, kind="ExternalInput").ap()
    wv_d = nc.dram_tensor("wv", [E, E], F32R, kind="ExternalInput").ap()
    wo_d = nc.dram_tensor("wo", [E, E], F32R, kind="ExternalInput").ap()
    if with_bias:
        bq_d = nc.dram_tensor("bq", [128, EC], F32, kind="ExternalInput").ap()
        bk_d = nc.dram_tensor("bk", [128, EC], F32, kind="ExternalInput").ap()
        bo_d = nc.dram_tensor("bo", [1, E], F32, kind="ExternalInput").ap()

    out_d = nc.dram_tensor("out", [SQL, E], F32, kind="ExternalOutput").ap()
    mat_d = nc.dram_tensor("mattn", [SQL, SK], F32, kind="ExternalOutput").ap()

    # v natural spill, split by d-half so pairs 0-3 only wait on half 0
    vd = [nc.dram_tensor(f"vd{i}", [SK, E // 2], F32R).ap() for i in range(2)]
    rdend = nc.dram_tensor("rdend", [H, SQL], F32R).ap()  # 1/denom rows

    with tile.TileContext(nc) as tc, tc.tile_pool(name="pin", bufs=1) as pin:
        # per-chunk resident tiles -> fine-grained deps, attention overlaps
        # the projection tail
        qT = [pin.tile([128, SQL], F32R, name=f"qT{c}", tag=f"qT{c}")
              for c in range(DC)]
        kT = [pin.tile([128, SK], F32R, name=f"kT{c}", tag=f"kT{c}")
              for c in range(DC)]
        rden4 = pin.tile([128, SQJ, H], F32)        # 1/denom per (sq, head)
        ident = pin.tile([128, 128], F32)
        make_identity(nc, ident)
        if with_bias:
            bqs = pin.tile([128, EC], F32)
            bks = pin.tile([128, EC], F32)
            bob = pin.tile([128, E], F32)
            nc.sync.dma_start(out=bqs, in_=bq_d)
            nc.sync.dma_start(out=bks, in_=bk_d)
            nc.gpsimd.dma_start(out=bob, in_=bcast_rows(bo_d[0:1, :], 128))

        # ---------------- P1: qT ----------------
        with tc.tile_pool(name="pq", bufs=1) as pq, \
             tc.tile_pool(name="ppq", bufs=4, space="PSUM") as ppq:
            xq = [pq.tile([128, SQL], F32R, name=f"xq{e}", tag=f"xq{e}")
                  for e in range(EC)]
            wq = [pq.tile([128, E], F32R, name=f"wq{e}", tag=f"wq{e}")
                  for e in range(EC)]
            for e in range(EC):
                # contiguous row-block loads: 2-4 KiB inner runs
                nc.sync.dma_start(out=xq[e], in_=xq_d[128 * e:128 * (e + 1), :])
                nc.sync.dma_start(out=wq[e], in_=wq_d[128 * e:128 * (e + 1), :])
            for c in range(DC):
                ps = ppq.tile([128, SQL], F32)
                for e in range(EC):
                    nc.tensor.matmul(
                        ps, wq[e][:, 128 * c:128 * (c + 1)], xq[e],
                        start=(e == 0), stop=(e == EC - 1),
                    )
                if with_bias:
                    nc.vector.tensor_scalar_add(qT[c], ps, bqs[:, c:c + 1])
                else:
                    nc.vector.tensor_copy(qT[c], ps)

        # ---------------- P2: kT (resident) ----------------
        with tc.tile_pool(name="pk", bufs=1) as pk, \
             tc.tile_pool(name="ppk", bufs=4, space="PSUM") as ppk:
            xk = [pk.tile([128, SK], F32R, name=f"xk{e}", tag=f"xk{e}")
                  for e in range(EC)]
            wk = [pk.tile([128, E], F32R, name=f"wk{e}", tag=f"wk{e}")
                  for e in range(EC)]
            for e in range(EC):
                nc.sync.dma_start(out=xk[e], in_=xk_d[128 * e:128 * (e + 1), :])
                nc.sync.dma_start(out=wk[e], in_=wk_d[128 * e:128 * (e + 1), :])
            for c in range(DC):
                for t in range(SKT5):
                    ps = ppk.tile([128, 512], F32)
                    for e in range(EC):
                        nc.tensor.matmul(
                            ps, wk[e][:, 128 * c:128 * (c + 1)],
                            xk[e][:, 512 * t:512 * (t + 1)],
                            start=(e == 0), stop=(e == EC - 1),
                        )
                    dst = kT[c][:, 512 * t:512 * (t + 1)]
                    if with_bias:
                        nc.vector.tensor_scalar_add(dst, ps, bks[:, c:c + 1])
                    else:
                        nc.vector.tensor_copy(dst, ps)

        # ------------- P3: v (natural) -> DRAM, d-half outer -------------
        with tc.tile_pool(name="pv", bufs=1) as pv, \
             tc.tile_pool(name="pvs", bufs=4) as pvs, \
             tc.tile_pool(name="ppv", bufs=4, space="PSUM") as ppv:
            xv = [pv.tile([128, SK], F32R, name=f"xv{e}", tag=f"xv{e}")
                  for e in range(EC)]
            wv = [pv.tile([128, E], F32R, name=f"wv{e}", tag=f"wv{e}")
                  for e in range(EC)]
            for e in range(EC):
                # gate P3 loads behind qT so P1/P2 loads get HBM BW first
                nc.vector.tensor_copy(xv[e][0:1, 0:1], qT[0][0:1, 0:1])
                nc.vector.tensor_copy(wv[e][0:1, 0:1], qT[0][0:1, 0:1])
                nc.sync.dma_start(out=xv[e], in_=xv_d[128 * e:128 * (e + 1), :])
                nc.sync.dma_start(out=wv[e], in_=wv_d[128 * e:128 * (e + 1), :])
            for dt_ in range(2):           # d-half outer: half 0 finishes first
                for s in range(SKT):       # 16 sk chunks of 128
                    ps = ppv.tile([128, 512], F32)
                    for e in range(EC):
                        nc.tensor.matmul(
                            ps, xv[e][:, 128 * s:128 * (s + 1)],
                            wv[e][:, 512 * dt_:512 * (dt_ + 1)],
                            start=(e == 0), stop=(e == EC - 1),
                        )
                    st = pvs.tile([128, 512], F32R)
                    nc.vector.tensor_copy(st, ps)
                    nc.sync.dma_start(
                        out=vd[dt_][128 * s:128 * (s + 1), :], in_=st)

        # ---------------- attention ----------------
        with tc.tile_pool(name="ppost", bufs=1) as ppost:
            mean_acc = ppost.tile([128, SQJ, SK], F32)   # 4 MiB
            attT = ppost.tile([128, DC, SQL], F32R)      # 2 MiB

            with tc.tile_pool(name="pa", bufs=2) as pa, \
                 tc.tile_pool(name="pe1", bufs=3) as pe1, \
                 tc.tile_pool(name="pe2", bufs=2) as pe2, \
                 tc.tile_pool(name="pn", bufs=2) as pn, \
                 tc.tile_pool(name="psc", bufs=2, space="PSUM") as psc, \
                 tc.tile_pool(name="pat", bufs=1, space="PSUM") as pat, \
                 tc.tile_pool(name="psm", bufs=4) as psm:
                for p in range(NP):
                    h0, h1 = 2 * p, 2 * p + 1
                    dh, dcol = p // 4, (p % 4) * 128
                    v2 = pa.tile([128, SKT, 128], F32R, tag="v2")
                    nc.sync.dma_start(
                        out=v2,
                        in_=vd[dh][:, dcol:dcol + 128].rearrange(
                            "(t q) d -> q t d", q=128))
                    qTp = qT[p]
                    kTp = kT[p]

                    # transposed pass: scoresT -> exp -> attendedT
                    ps_att0 = pat.tile([64, SQL], F32, tag="att0")
                    ps_att1 = pat.tile([64, SQL], F32, tag="att1")
                    for t in range(SKT):
                        ps2 = psc.tile([128, 2 * SQL], F32, tag="sA")
                        nc.tensor.matmul(
                            ps2[:, 0:SQL], kTp[0:64, 128 * t:128 * (t + 1)],
                            qTp[0:64, :], start=True, stop=True)
                        nc.tensor.matmul(
                            ps2[:, SQL:2 * SQL],
                            kTp[64:128, 128 * t:128 * (t + 1)],
                            qTp[64:128, :], start=True, stop=True)
                        ex2 = pe1.tile([128, 2 * SQL], F32R, tag="ex")
                        nc.scalar.activation(ex2, ps2, ACTF.Exp)
                        nc.tensor.matmul(
                            ps_att0, v2[:, t, 0:64], ex2[:, 0:SQL],
                            start=(t == 0), stop=(t == SKT - 1))
                        nc.tensor.matmul(
                            ps_att1, v2[:, t, 64:128], ex2[:, SQL:2 * SQL],
                            start=(t == 0), stop=(t == SKT - 1))

                    # natural pass: scores -> exp(+rowsum) -> mean_acc
                    for j in range(SQJ):
                        enat0 = pe2.tile([128, SK], F32, tag="en0")
                        enat1 = pe2.tile([128, SK], F32, tag="en1")
                        acc0 = psm.tile([128, SKT5], F32, tag="acc0")
                        acc1 = psm.tile([128, SKT5], F32, tag="acc1")
                        for u in range(SKT5):
                            pna = psc.tile([128, SQL], F32, tag="sA")
                            pnb = psc.tile([128, SQL], F32, tag="sB")
                            nc.tensor.matmul(
                                pna,
                                qTp[0:64, 128 * j:128 * (j + 1)],
                                kTp[0:64, 512 * u:512 * (u + 1)],
                                start=True, stop=True)
                            nc.tensor.matmul(
                                pnb,
                                qTp[64:128, 128 * j:128 * (j + 1)],
                                kTp[64:128, 512 * u:512 * (u + 1)],
                                start=True, stop=True)
                            nc.scalar.activation(
                                enat0[:, 512 * u:512 * (u + 1)], pna,
                                ACTF.Exp, accum_out=acc0[:, u:u + 1])
                            nc.scalar.activation(
                                enat1[:, 512 * u:512 * (u + 1)], pnb,
                                ACTF.Exp, accum_out=acc1[:, u:u + 1])
                        for h, enat, acc in ((h0, enat0, acc0),
                                             (h1, enat1, acc1)):
                            den = psm.tile([128, 1], F32, tag="den")
                            nc.vector.tensor_reduce(den, acc, AX.X, ALU.add)
                            nc.vector.reciprocal(rden4[:, j, h:h + 1], den)
                            rd16 = psm.tile([128, 1], F32, tag="rd16")
                            nc.vector.tensor_scalar_mul(
                                rd16, rden4[:, j, h:h + 1], 1.0 / H)
                            if h == 0:
                                nc.vector.tensor_scalar(
                                    mean_acc[:, j, :], enat, rd16, None,
                                    op0=ALU.mult)
                            else:
                                nc.vector.scalar_tensor_tensor(
                                    mean_acc[:, j, :], enat, rd16,
                                    mean_acc[:, j, :],
                                    op0=ALU.mult, op1=ALU.add)

                    nc.vector.tensor_copy(attT[0:64, p, :], ps_att0)
                    nc.vector.tensor_copy(attT[64:128, p, :], ps_att1)
                for j in range(SQJ):
                    nc.sync.dma_start(
                        out=mat_d[128 * j:128 * (j + 1), :], in_=mean_acc[:, j, :])

                # normalize attT: transpose denoms -> DRAM -> row broadcast
                rdenT = pn.tile([16, SQL], F32R, tag="rdenT")
                for j in range(SQJ):
                    pt = psc.tile([16, 128], F32, tag="sA")
                    nc.tensor.transpose(pt, rden4[:, j, :], ident)
                    nc.vector.tensor_copy(rdenT[:, 128 * j:128 * (j + 1)], pt)
                nc.sync.dma_start(out=rdend, in_=rdenT)
                for p in range(NP):
                    bc = pn.tile([128, SQL], F32R, tag="bc")
                    nc.sync.dma_start(
                        out=bc[0:64, :],
                        in_=bcast_rows(rdend[2 * p:2 * p + 1, :], 64))
                    nc.sync.dma_start(
                        out=bc[64:128, :],
                        in_=bcast_rows(rdend[2 * p + 1:2 * p + 2, :], 64))
                    nc.vector.tensor_mul(attT[:, p, :], attT[:, p, :], bc)

            # ---------------- output projection ----------------
            with tc.tile_pool(name="po", bufs=1) as po, \
                 tc.tile_pool(name="pob", bufs=4) as pob, \
                 tc.tile_pool(name="ppo", bufs=4, space="PSUM") as ppo:
                wo = [po.tile([128, E], F32R, name=f"wo{c}", tag=f"wo{c}")
                      for c in range(DC)]
                for c in range(DC):
                    nc.vector.tensor_copy(wo[c][0:1, 0:1], kT[7][0:1, 0:1])
                    nc.sync.dma_start(
                        out=wo[c], in_=wo_d[128 * c:128 * (c + 1), :])
                for m in range(SQJ):
                    for n in range(2):
                        ps = ppo.tile([128, 512], F32)
                        for c in range(DC):
                            nc.tensor.matmul(
                                ps, attT[:, c, 128 * m:128 * (m + 1)],
                                wo[c][:, 512 * n:512 * (n + 1)],
                                start=(c == 0), stop=(c == DC - 1))
                        so = pob.tile([128, 512], F32)
                        if with_bias:
                            nc.vector.tensor_add(
                                so, ps, bob[:, 512 * n:512 * (n + 1)])
                        else:
                            nc.vector.tensor_copy(so, ps)
                        nc.sync.dma_start(
                            out=out_d[128 * m:128 * (m + 1),
                                      512 * n:512 * (n + 1)],
                            in_=so)

    nc.compile()
    return nc


_NC_CACHE = {}


def _get_nc(with_bias: bool):
    if with_bias not in _NC_CACHE:
        _NC_CACHE[with_bias] = build_kernel(with_bias)
    return _NC_CACHE[with_bias]


def kernel(query, key, value, Wq, bq, Wk, bk, Wv, bv, Wo, bo, _trace=False):
    query = np.asarray(query, np.float32)
    key = np.asarray(key, np.float32)
    value = np.asarray(value, np.float32)
    Wq = np.asarray(Wq, np.float32)
    Wk = np.asarray(Wk, np.float32)
    Wv = np.asarray(Wv, np.float32)
    Wo = np.asarray(Wo, np.float32)
    bq = np.asarray(bq, np.float32)
    bk = np.asarray(bk, np.float32)
    bv = np.asarray(bv, np.float32)
    bo = np.asarray(bo, np.float32)

    with_bias = bool(bq.any() or bk.any() or bv.any() or bo.any())
    nc = _get_nc(with_bias)

    # weights, shared by all cores
    shared = {
        "wq": np.ascontiguousarray((SCALE * Wq).T),   # [E(in), E(out d)]
        "wk": np.ascontiguousarray(Wk.T),
        "wv": np.ascontiguousarray(Wv.T),
        "wo": np.ascontiguousarray(Wo.T),             # [d, e]
    }
    if with_bias:
        bo_eff = bo + Wo @ bv                         # fold v-bias into out-bias
        shared["bq"] = np.ascontiguousarray((SCALE * bq).reshape(EC, 128).T)
        shared["bk"] = np.ascontiguousarray(bk.reshape(EC, 128).T)
        shared["bo"] = np.ascontiguousarray(bo_eff.reshape(1, E))

    in_maps = []
    for c in range(NCORES):
        b, half = c // 2, c % 2
        s0 = half * SQL
        m = dict(shared)
        m["xq"] = np.ascontiguousarray(query[b, s0:s0 + SQL, :].T)
        m["xk"] = np.ascontiguousarray(key[b].T)
        m["xv"] = np.ascontiguousarray(value[b].T)
        in_maps.append(m)

    res = run_bass_kernel_spmd(nc, in_maps, list(range(NCORES)), trace=_trace)

    output = np.empty((B, SQ, E), np.float32)
    mean_attn = np.empty((B, SQ, SK), np.float32)
    for c in range(NCORES):
        b, half = c // 2, c % 2
        s0 = half * SQL
        output[b, s0:s0 + SQL, :] = res.results[c]["out"]
        mean_attn[b, s0:s0 + SQL, :] = res.results[c]["mattn"]

    if _trace:
        kernel.last_results = res
    return output, mean_attn


# revision 26
# speedup vs baseline: 1.1117x; 1.1117x over previous
"""Multi-head cross-attention Bass/Tile kernel for Trainium2, SPMD over 8 cores.

Problem: B=4, SQ=1024, SK=2048, E=1024, H=16 heads, Dh=64.
  q = query @ Wq.T + bq ; k = key @ Wk.T + bk ; v = value @ Wv.T + bv
  attn = softmax(SCALE * q k^T) ; attended = attn @ v
  output = attended @ Wo.T + bo ; mean_attn = attn.mean(heads)

Sharding: 8 cores = (batch, query-half) shards. Each core owns one batch and
512 query rows, computes ALL heads for that shard, so both outputs are disjoint
slices -> host unshard is pure concatenation (no reduction).

Per-core pipeline (matmuls in fp32r = full-rate TF32-like, ~4e-4 rel err):
  P1: qT[d,sq] = (SCALE*Wq) @ queryT (+SCALE*bq)   [SBUF resident, 8 chunk tiles]
  P2: kT[d,sk] = Wk @ keyT (+bk)                   [SBUF resident, 8 chunk tiles]
  P3: v[sk,d]  = (valueT stationary) @ WvT         [DRAM spill, 2 d-half tensors]
  Per head pair p (heads 2p, 2p+1 ride distinct PE row groups -> concurrent):
    transposed pass: scoresT[sk,sq] -> exp (ACT, PSUM src) -> attendedT
      accumulated over sk chunks (PE)
    natural pass: scores[sq,sk] -> exp with accum_out = row sums (ACT)
      -> 1/denom (DVE) ; mean_acc[j] += exp * 1/(16*denom) (DVE fused STT)
    attendedT *= 1/denom (PE-transposed denoms -> DRAM row bcast -> DVE mul)
  output = attendedT^T @ WoT (+ bo + Wo@bv folded host-side)

Chunk-granular tiles (kT/qT per d-chunk, vd per d-half) let the Tile scheduler
overlap head-pair attention with the tail of the projections.

Self-contained: hardcodes shapes; no sibling imports.
"""

import numpy as np

import concourse.bass as bass
import concourse.tile as tile
from concourse import bacc, mybir
from concourse.bass_utils import run_bass_kernel_spmd
from concourse.masks import make_identity

F32 = mybir.dt.float32
F32R = mybir.dt.float32r
AX = mybir.AxisListType
ALU = mybir.AluOpType
ACTF = mybir.ActivationFunctionType

B, SQ, SK, E, H, DH = 4, 1024, 2048, 1024, 16, 64
SCALE = DH ** -0.5
NCORES = 8
SQL = SQ // 2           # 512 query rows per core
EC = E // 128           # 8 embed chunks
DC = E // 128           # 8 d chunks (qkv output dim = E)
NP = H // 2             # 8 head pairs
SKT = SK // 128         # 16 sk tiles of 128
SKT5 = SK // 512        # 4 sk tiles of 512
SQJ = SQL // 128        # 4 sq tiles of 128


def bcast_rows(src_row_ap, nrows):
    """AP reading one DRAM row broadcast across nrows partitions."""
    return bass.AP(src_row_ap.tensor, src_row_ap.offset,
                   [[0, nrows]] + list(src_row_ap.ap[1:]))


def build_kernel(with_bias: bool):
    nc = bacc.Bacc("TRN2", target_bir_lowering=False, debug=False)

    # ---- DRAM I/O ----
    xq_d = nc.dram_tensor("xq", [E, SQL], F32R, kind="ExternalInput").ap()
    xk_d = nc.dram_tensor("xk", [E, SK], F32R, kind="ExternalInput").ap()
    xv_d = nc.dram_tensor("xv", [E, SK], F32R, kind="ExternalInput").ap()
    wq_d = nc.dram_tensor("wq", [E, E], F32R, kind="ExternalInput").ap()
    wk_d = nc.dram_tensor("wk", [E, E], F32R, kind="ExternalInput").ap()
    wv_d = nc.dram_tensor("wv", [E, E], F32R, kind="ExternalInput").ap()
    wo_d = nc.dram_tensor("wo", [E, E], F32R, kind="ExternalInput").ap()
    if with_bias:
        bq_d = nc.dram_tensor("bq", [128, EC], F32, kind="ExternalInput").ap()
        bk_d = nc.dram_tensor("bk", [128, EC], F32, kind="ExternalInput").ap()
        bo_d = nc.dram_tensor("bo", [1, E], F32, kind="ExternalInput").ap()

    out_d = nc.dram_tensor("out", [SQL, E], F32, kind="ExternalOutput").ap()
    mat_d = nc.dram_tensor("mattn", [SQL, SK], F32, kind="ExternalOutput").ap()

    # v natural spill, split by d-half so pairs 0-3 only wait on half 0
    vd = [nc.dram_tensor(f"vd{i}", [SK, E // 2], F32R).ap() for i in range(2)]
    rdend = nc.dram_tensor("rdend", [H, SQL], F32R).ap()  # 1/denom rows

    with tile.TileContext(nc) as tc, tc.tile_pool(name="pin", bufs=1) as pin:
        # per-chunk resident tiles -> fine-grained deps, attention overlaps
        # the projection tail
        qT = [pin.tile([128, SQL], F32R, name=f"qT{c}", tag=f"qT{c}")
              for c in range(DC)]
        kT = [pin.tile([128, SK], F32R, name=f"kT{c}", tag=f"kT{c}")
              for c in range(DC)]
        rden4 = pin.tile([128, SQJ, H], F32)        # 1/denom per (sq, head)
        ident = pin.tile([128, 128], F32)
        make_identity(nc, ident)
        if with_bias:
            bqs = pin.tile([128, EC], F32)
            bks = pin.tile([128, EC], F32)
            bob = pin.tile([128, E], F32)
            nc.sync.dma_start(out=bqs, in_=bq_d)
            nc.sync.dma_start(out=bks, in_=bk_d)
            nc.gpsimd.dma_start(out=bob, in_=bcast_rows(bo_d[0:1, :], 128))

        # ---------------- P1: qT ----------------
        with tc.tile_pool(name="pq", bufs=1) as pq, \
             tc.tile_pool(name="ppq", bufs=4, space="PSUM") as ppq:
            xq = [pq.tile([128, SQL], F32R, name=f"xq{e}", tag=f"xq{e}")
                  for e in range(EC)]
            wq = [pq.tile([128, E], F32R, name=f"wq{e}", tag=f"wq{e}")
                  for e in range(EC)]
            for e in range(EC):
                # contiguous row-block loads: 2-4 KiB inner runs
                nc.sync.dma_start(out=xq[e], in_=xq_d[128 * e:128 * (e + 1), :])
                nc.sync.dma_start(out=wq[e], in_=wq_d[128 * e:128 * (e + 1), :])
            for c in range(DC):
                ps = ppq.tile([128, SQL], F32)
                for e in range(EC):
                    nc.tensor.matmul(
                        ps, wq[e][:, 128 * c:128 * (c + 1)], xq[e],
                        start=(e == 0), stop=(e == EC - 1),
                    )
                if with_bias:
                    nc.vector.tensor_scalar_add(qT[c], ps, bqs[:, c:c + 1])
                else:
                    nc.vector.tensor_copy(qT[c], ps)

        # ---------------- P2: kT (resident) ----------------
        with tc.tile_pool(name="pk", bufs=1) as pk, \
             tc.tile_pool(name="ppk", bufs=4, space="PSUM") as ppk:
            xk = [pk.tile([128, SK], F32R, name=f"xk{e}", tag=f"xk{e}")
                  for e in range(EC)]
            wk = [pk.tile([128, E], F32R, name=f"wk{e}", tag=f"wk{e}")
                  for e in range(EC)]
            for e in range(EC):
                nc.sync.dma_start(out=xk[e], in_=xk_d[128 * e:128 * (e + 1), :])
                nc.sync.dma_start(out=wk[e], in_=wk_d[128 * e:128 * (e + 1), :])
            for c in range(DC):
                for t in range(SKT5):
                    ps = ppk.tile([128, 512], F32)
                    for e in range(EC):
                        nc.tensor.matmul(
                            ps, wk[e][:, 128 * c:128 * (c + 1)],
                            xk[e][:, 512 * t:512 * (t + 1)],
                            start=(e == 0), stop=(e == EC - 1),
                        )
                    dst = kT[c][:, 512 * t:512 * (t + 1)]
                    if with_bias:
                        nc.vector.tensor_scalar_add(dst, ps, bks[:, c:c + 1])
                    else:
                        nc.vector.tensor_copy(dst, ps)

        # ------------- P3: v (natural) -> DRAM, d-half outer -------------
        with tc.tile_pool(name="pv", bufs=1) as pv, \
             tc.tile_pool(name="pvs", bufs=4) as pvs, \
             tc.tile_pool(name="ppv", bufs=4, space="PSUM") as ppv:
            xv = [pv.tile([128, SK], F32R, name=f"xv{e}", tag=f"xv{e}")
                  for e in range(EC)]
            wv = [pv.tile([128, E], F32R, name=f"wv{e}", tag=f"wv{e}")
                  for e in range(EC)]
            for e in range(EC):
                # gate P3 loads behind qT so P1/P2 loads get HBM BW first
                nc.vector.tensor_copy(xv[e][0:1, 0:1], qT[0][0:1, 0:1])
                nc.vector.tensor_copy(wv[e][0:1, 0:1], qT[0][0:1, 0:1])
                nc.sync.dma_start(out=xv[e], in_=xv_d[128 * e:128 * (e + 1), :])
                nc.sync.dma_start(out=wv[e], in_=wv_d[128 * e:128 * (e + 1), :])
            for dt_ in range(2):           # d-half outer: half 0 finishes first
                for s in range(SKT):       # 16 sk chunks of 128
                    ps = ppv.tile([128, 512], F32)
                    for e in range(EC):
                        nc.tensor.matmul(
                            ps, xv[e][:, 128 * s:128 * (s + 1)],
                            wv[e][:, 512 * dt_:512 * (dt_ + 1)],
                            start=(e == 0), stop=(e == EC - 1),
                        )
                    st = pvs.tile([128, 512], F32R)
                    nc.vector.tensor_copy(st, ps)
                    nc.sync.dma_start(
                        out=vd[dt_][128 * s:128 * (s + 1), :], in_=st)

        # ---------------- attention ----------------
        with tc.tile_pool(name="ppost", bufs=1) as ppost:
            mean_acc = ppost.tile([128, SQJ, SK], F32)   # 4 MiB
            attT = ppost.tile([128, DC, SQL], F32R)      # 2 MiB

            with tc.tile_pool(name="pa", bufs=2) as pa, \
                 tc.tile_pool(name="pe1", bufs=3) as pe1, \
                 tc.tile_pool(name="pe2", bufs=2) as pe2, \
                 tc.tile_pool(name="pn", bufs=2) as pn, \
                 tc.tile_pool(name="psc", bufs=3, space="PSUM") as psc, \
                 tc.tile_pool(name="pat", bufs=1, space="PSUM") as pat, \
                 tc.tile_pool(name="psm", bufs=4) as psm:
                for p in range(NP):
                    h0, h1 = 2 * p, 2 * p + 1
                    dh, dcol = p // 4, (p % 4) * 128
                    v2 = pa.tile([128, SKT, 128], F32R, tag="v2")
                    nc.sync.dma_start(
                        out=v2,
                        in_=vd[dh][:, dcol:dcol + 128].rearrange(
                            "(t q) d -> q t d", q=128))
                    qTp = qT[p]
                    kTp = kT[p]

                    # transposed pass: scoresT -> exp -> attendedT
                    ps_att0 = pat.tile([64, SQL], F32, tag="att0")
                    ps_att1 = pat.tile([64, SQL], F32, tag="att1")
                    for t in range(SKT):
                        ps2 = psc.tile([128, 2 * SQL], F32, tag="sA")
                        nc.tensor.matmul(
                            ps2[:, 0:SQL], kTp[0:64, 128 * t:128 * (t + 1)],
                            qTp[0:64, :], start=True, stop=True)
                        nc.tensor.matmul(
                            ps2[:, SQL:2 * SQL],
                            kTp[64:128, 128 * t:128 * (t + 1)],
                            qTp[64:128, :], start=True, stop=True)
                        ex2 = pe1.tile([128, 2 * SQL], F32R, tag="ex")
                        nc.scalar.activation(ex2, ps2, ACTF.Exp)
                        nc.tensor.matmul(
                            ps_att0, v2[:, t, 0:64], ex2[:, 0:SQL],
                            start=(t == 0), stop=(t == SKT - 1))
                        nc.tensor.matmul(
                            ps_att1, v2[:, t, 64:128], ex2[:, SQL:2 * SQL],
                            start=(t == 0), stop=(t == SKT - 1))

                    # natural pass: scores -> exp(+rowsum) -> mean_acc.
                    # Two N=512 matmuls fill each [128,1024] PSUM tile so the
                    # exp runs as one wide ACT call (halves ACT call+accum
                    # overhead vs per-512 calls).
                    for j in range(SQJ):
                        enat0 = pe2.tile([128, SK], F32, tag="en0")
                        enat1 = pe2.tile([128, SK], F32, tag="en1")
                        acc0 = psm.tile([128, 2], F32, tag="acc0")
                        acc1 = psm.tile([128, 2], F32, tag="acc1")
                        for u in range(2):
                            pna = psc.tile([128, 2 * SQL], F32, tag="sA")
                            pnb = psc.tile([128, 2 * SQL], F32, tag="sA")
                            for g in range(2):
                                sk0 = 1024 * u + 512 * g
                                nc.tensor.matmul(
                                    pna[:, 512 * g:512 * (g + 1)],
                                    qTp[0:64, 128 * j:128 * (j + 1)],
                                    kTp[0:64, sk0:sk0 + 512],
                                    start=True, stop=True)
                                nc.tensor.matmul(
                                    pnb[:, 512 * g:512 * (g + 1)],
                                    qTp[64:128, 128 * j:128 * (j + 1)],
                                    kTp[64:128, sk0:sk0 + 512],
                                    start=True, stop=True)
                            nc.scalar.activation(
                                enat0[:, 1024 * u:1024 * (u + 1)], pna,
                                ACTF.Exp, accum_out=acc0[:, u:u + 1])
                            nc.scalar.activation(
                                enat1[:, 1024 * u:1024 * (u + 1)], pnb,
                                ACTF.Exp, accum_out=acc1[:, u:u + 1])
                        for h, enat, acc in ((h0, enat0, acc0),
                                             (h1, enat1, acc1)):
                            den = psm.tile([128, 1], F32, tag="den")
                            nc.vector.tensor_reduce(den, acc, AX.X, ALU.add)
                            nc.vector.reciprocal(rden4[:, j, h:h + 1], den)
                            rd16 = psm.tile([128, 1], F32, tag="rd16")
                            nc.vector.tensor_scalar_mul(
                                rd16, rden4[:, j, h:h + 1], 1.0 / H)
                            if h == 0:
                                nc.vector.tensor_scalar(
                                    mean_acc[:, j, :], enat, rd16, None,
                                    op0=ALU.mult)
                            else:
                                nc.vector.scalar_tensor_tensor(
                                    mean_acc[:, j, :], enat, rd16,
                                    mean_acc[:, j, :],
                                    op0=ALU.mult, op1=ALU.add)

                    nc.vector.tensor_copy(attT[0:64, p, :], ps_att0)
                    nc.vector.tensor_copy(attT[64:128, p, :], ps_att1)
                for j in range(SQJ):
                    nc.sync.dma_start(
                        out=mat_d[128 * j:128 * (j + 1), :], in_=mean_acc[:, j, :])

                # normalize attT: transpose denoms -> DRAM -> row broadcast
                rdenT = pn.tile([16, SQL], F32R, tag="rdenT")
                for j in range(SQJ):
                    pt = psc.tile([16, 128], F32, tag="sA")
                    nc.tensor.transpose(pt, rden4[:, j, :], ident)
                    nc.vector.tensor_copy(rdenT[:, 128 * j:128 * (j + 1)], pt)
                nc.sync.dma_start(out=rdend, in_=rdenT)
                for p in range(NP):
                    bc = pn.tile([128, SQL], F32R, tag="bc")
                    nc.sync.dma_start(
                        out=bc[0:64, :],
                        in_=bcast_rows(rdend[2 * p:2 * p + 1, :], 64))
                    nc.sync.dma_start(
                        out=bc[64:128, :],
                        in_=bcast_rows(rdend[2 * p + 1:2 * p + 2, :], 64))
                    nc.vector.tensor_mul(attT[:, p, :], attT[:, p, :], bc)

            # ---------------- output projection ----------------
            with tc.tile_pool(name="po", bufs=1) as po, \
                 tc.tile_pool(name="pob", bufs=4) as pob, \
                 tc.tile_pool(name="ppo", bufs=4, space="PSUM") as ppo:
                wo = [po.tile([128, E], F32R, name=f"wo{c}", tag=f"wo{c}")
                      for c in range(DC)]
                for c in range(DC):
                    nc.vector.tensor_copy(wo[c][0:1, 0:1], kT[7][0:1, 0:1])
                    nc.sync.dma_start(
                        out=wo[c], in_=wo_d[128 * c:128 * (c + 1), :])
                for m in range(SQJ):
                    for n in range(2):
                        ps = ppo.tile([128, 512], F32)
                        for c in range(DC):
                            nc.tensor.matmul(
                                ps, attT[:, c, 128 * m:128 * (m + 1)],
                                wo[c][:, 512 * n:512 * (n + 1)],
                                start=(c == 0), stop=(c == DC - 1))
                        so = pob.tile([128, 512], F32)
                        if with_bias:
                            nc.vector.tensor_add(
                                so, ps, bob[:, 512 * n:512 * (n + 1)])
                        else:
                            nc.vector.tensor_copy(so, ps)
                        nc.sync.dma_start(
                            out=out_d[128 * m:128 * (m + 1),
                                      512 * n:512 * (n + 1)],
                            in_=so)

    nc.compile()
    return nc


_NC_CACHE = {}


def _get_nc(with_bias: bool):
    if with_bias not in _NC_CACHE:
        _NC_CACHE[with_bias] = build_kernel(with_bias)
    return _NC_CACHE[with_bias]


def kernel(query, key, value, Wq, bq, Wk, bk, Wv, bv, Wo, bo, _trace=False):
    query = np.asarray(query, np.float32)
    key = np.asarray(key, np.float32)
    value = np.asarray(value, np.float32)
    Wq = np.asarray(Wq, np.float32)
    Wk = np.asarray(Wk, np.float32)
    Wv = np.asarray(Wv, np.float32)
    Wo = np.asarray(Wo, np.float32)
    bq = np.asarray(bq, np.float32)
    bk = np.asarray(bk, np.float32)
    bv = np.asarray(bv, np.float32)
    bo = np.asarray(bo, np.float32)

    with_bias = bool(bq.any() or bk.any() or bv.any() or bo.any())
    nc = _get_nc(with_bias)

    # weights, shared by all cores
    shared = {
        "wq": np.ascontiguousarray((SCALE * Wq).T),   # [E(in), E(out d)]
        "wk": np.ascontiguousarray(Wk.T),
        "wv": np.ascontiguousarray(Wv.T),
        "wo": np.ascontiguousarray(Wo.T),             # [d, e]
    }
    if with_bias:
        bo_eff = bo + Wo @ bv                         # fold v-bias into out-bias
        shared["bq"] = np.ascontiguousarray((SCALE * bq).reshape(EC, 128).T)
        shared["bk"] = np.ascontiguousarray(bk.reshape(EC, 128).T)
        shared["bo"] = np.ascontiguousarray(bo_eff.reshape(1, E))

    in_maps = []
    for c in range(NCORES):
        b, half = c // 2, c % 2
        s0 = half * SQL
        m = dict(shared)
        m["xq"] = np.ascontiguousarray(query[b, s0:s0 + SQL, :].T)
        m["xk"] = np.ascontiguousarray(key[b].T)
        m["xv"] = np.ascontiguousarray(value[b].T)
        in_maps.append(m)

    res = run_bass_kernel_spmd(nc, in_maps, list(range(NCORES)), trace=_trace)

    output = np.empty((B, SQ, E), np.float32)
    mean_attn = np.empty((B, SQ, SK), np.float32)
    for c in range(NCORES):
        b, half = c // 2, c % 2
        s0 = half * SQL
        output[b, s0:s0 + SQL, :] = res.results[c]["out"]
        mean_attn[b, s0:s0 + SQL, :] = res.results[c]["mattn"]

    if _trace:
        kernel.last_results = res
    return output, mean_attn
